# revision 7
# baseline (speedup 1.0000x reference)
"""Trainium2 Bass kernel for nn_BertClassifier_77309411685 (V9).

Data-parallel over 8 NeuronCores: each core handles 256 samples; the small
base linear and 12 expert heads are replicated.

V10 strategy (V8/V9 trace forensics: DVE scalar_tensor_tensor runs at ~1us per
768-elem op - 4x slower than tensor_scalar - so the all-Vector masked-mean
chain serialized 14us; DMA engines serve queues roughly FIFO at ~400GB/s
aggregate; indirect-DMA descriptor gen costs ~1.1us per op on GpSimd):
  * samples per core are permuted by span length (host-side; un-permuted on
    host): group B (g=0) = 128 longest spans (JB rows), group A (g=1) =
    128 shortest (JA rows).  Cuts gather bytes ~20% and PE mean work.
  * group B mean on the PE (V7 diag-stationary scheme, 0.84ns/col - the
    fastest engine per byte); group A mean on Vector (tensor_scalar mults
    + one strided tensor_reduce over j).  This splits the ~8.4us of mean
    work across engines so the PE (the bottleneck at ~16us) stays fed.
  * wire order: ctx weights first (feeds ctx matmuls), B gather (feeds the
    long PE mean chain), A gather, wbT-center LAST (its consumers - the
    close matmuls - run last anyway; a dummy sync DMA reading gB2 delays
    its issue so it cannot steal wire from B/A).
  * wbT host-prearranged [128, KC*INNER] so weight DMAs are contiguous
    multi-KB descriptors; gathers use 2 pieces per group (6KB descriptors).
  * PE warm-up from a memset tile during the DMA ramp (HAM clock gate).
  * expert heads: all 12 experts at once, bias via ones row; per-sample
    is_equal select + strided reduce; one packed [128, 6] output DMA.
"""

import numpy as np
from contextlib import ExitStack

import concourse.bass as bass
import concourse.tile as tile
from concourse import bacc, mybir
from concourse.bass import IndirectOffsetOnAxis
from concourse.bass_utils import run_bass_kernel_spmd

F32 = mybir.dt.float32
F16 = mybir.dt.float16
I32 = mybir.dt.int32

B, S, H = 2048, 256, 768
INNER, NB_CTX, NB_EXPERTS, NB_LABELS = 256, 2, 12, 3
NCORES = 8
BC = B // NCORES             # 256 samples per core
F3H = (NB_CTX + 1) * H       # 2304
KC = F3H // 128              # 18 contraction chunks
HC = H // 128                # 6 chunks per feature block
NE = NB_EXPERTS * NB_LABELS  # 36
EROWS = BC * S               # rows in the per-core embedding tensor

# The reference picks 2 static context positions host-side with this exact rng.
CTX_IDX = [int(v) for v in np.random.default_rng(seed=0).choice(np.arange(S), size=NB_CTX)]

MUL = mybir.AluOpType.mult
ADD = mybir.AluOpType.add


def _build(JA, JB):
    """Build the per-core program for group row counts (JA, JB)."""
    J1A, J1B = (JA + 1) // 2, (JB + 1) // 2
    J2A, J2B = JA - J1A, JB - J1B
    MOFF = NE + 2                                # mask cols offset in c32

    nc = bacc.Bacc(
        "TRN2",
        target_bir_lowering=False,
        debug=False,
        enable_asserts=False,
        num_devices=NCORES,
    )
    embT = nc.dram_tensor("embT", [EROWS, H], F16, kind="ExternalInput").ap()
    # gidx cols: 0=B rows [0,J1B), 1=B rows [J1B,JB), 2=A rows [0,J1A),
    #            3=A rows [J1A,JA)
    gidx = nc.dram_tensor("gidx", [128, 4], I32, kind="ExternalInput").ap()
    # wbT pre-arranged: wbT[p, c*INNER + m] = W_base[m, c*128 + p]
    wbT = nc.dram_tensor("wbT", [128, KC * INNER], F16, kind="ExternalInput").ap()
    ctxT = nc.dram_tensor("ctxT", [128, NB_CTX * HC * 256], F16, kind="ExternalInput").ap()
    # c16: diag(1/len_B) [0:128) + identity [128:256) + wexpA + wexpB
    c16 = nc.dram_tensor("c16", [128, 256 + 2 * NE], F16, kind="ExternalInput").ap()
    # c32: io36 + categories-as-float (2) + mA*1/len (JA) + mB 0/1 (JB)
    #      + b_base (t p) layout (2)
    c32 = nc.dram_tensor("c32", [128, MOFF + JA + JB + 2], F32, kind="ExternalInput").ap()
    # c1: ones row [0:256) + expert bias row [256:292)
    c1 = nc.dram_tensor("c1", [1, 256 + NE], F16, kind="ExternalInput").ap()
    out = nc.dram_tensor("out", [128, 2 * NB_LABELS], F32, kind="ExternalOutput").ap()

    with tile.TileContext(nc) as tc, ExitStack() as ctx:
        pool = ctx.enter_context(tc.tile_pool(name="main", bufs=1))
        pst = ctx.enter_context(tc.tile_pool(name="pst", bufs=1, space="PSUM"))

        # --- tiny front-of-queue loads ---
        # gidx rides the Scalar queue FIRST: the gpsimd queue's tiny DMA
        # took ~4us to semaphore behind the jammed weight stream.
        gidx_t = pool.tile([128, 4], I32)
        nc.scalar.dma_start(gidx_t[:], gidx[:, :])

        c16_t = pool.tile([128, 256 + 2 * NE], F16)
        nc.scalar.dma_start(c16_t[:], c16[:, :])
        diagB = c16_t[:, 0:128]
        identity = c16_t[:, 128:256]
        wexpA = c16_t[:, 256:256 + NE]
        wexpB = c16_t[:, 256 + NE:256 + 2 * NE]
        c32_t = pool.tile([128, MOFF + JA + JB + 2], F32)
        nc.scalar.dma_start(c32_t[:], c32[:, :])
        io36f = c32_t[:, 0:NE]
        catf = c32_t[:, NE:NE + 2]
        bb_t = c32_t[:, MOFF + JA + JB:MOFF + JA + JB + 2]
        c1_t = pool.tile([1, 256 + NE], F16)
        nc.scalar.dma_start(c1_t[:], c1[:, :])
        ones1 = c1_t[:, 0:256]
        wexpC = c1_t[:, 256:256 + NE]

        # PE warm-up from a memset tile (no DMA dependency)
        warm_src = pool.tile([128, 512], F16)
        nc.vector.memset(warm_src[:], 0.0)
        warm = pst.tile([128, 512], F32, tag="warm", bufs=1)
        for w in range(5):
            nc.tensor.matmul(warm[:], lhsT=warm_src[:, 0:128], rhs=warm_src[:],
                             start=(w == 0), stop=(w == 4))

        # --- span gathers on the gpsimd SWDGE queue: B first (feeds the
        # long PE chain), A after ---
        def gather(col, rows, tag):
            gt = pool.tile([128, rows * H], F16, tag=tag, bufs=1)
            nc.gpsimd.indirect_dma_start(
                out=gt[:], out_offset=None, in_=embT,
                in_offset=IndirectOffsetOnAxis(ap=gidx_t[:, col:col + 1], axis=0),
            )
            return gt

        gB1 = gather(0, J1B, "gB1")
        gB2 = gather(1, J2B, "gB2") if J2B else None
        gA1 = gather(2, J1A, "gA1")
        gA2 = gather(3, J2A, "gA2") if J2A else None

        # --- weight streams on the Sync queue: ctx parts first; wbT-center
        # delayed behind a dummy DMA that reads gB2 so it issues only after
        # the B gather has drained the wire ---
        featT = pool.tile([128, KC * 256], F16)
        wbT_t = pool.tile([128, KC * INNER], F16)
        nc.sync.dma_start(wbT_t[:, HC * INNER:], wbT[:, HC * INNER:])
        nc.sync.dma_start(featT[:, HC * 256:], ctxT[:, :])
        nc.sync.dma_start(wbT_t[:, :HC * INNER], wbT[:, :HC * INNER])

        # --- Vector: build diag(mB_j/len) stationaries for the PE mean ---
        dmaskB = pool.tile([128, JB * 128], F16)
        for j in range(JB):
            nc.vector.tensor_scalar(
                dmaskB[:, j * 128:(j + 1) * 128], diagB,
                c32_t[:, MOFF + JA + j:MOFF + JA + j + 1], None, op0=MUL)

        # --- PE: ctx chunks (paced by the weight stream) ---
        accs = [[pst.tile([128, 128], F32, tag=f"acc{g}{mt}", bufs=1,
                          name=f"acc{g}{mt}") for mt in range(2)]
                for g in range(2)]
        for c in range(HC, KC):
            for g in range(2):
                for mt in range(2):
                    nc.tensor.matmul(
                        accs[g][mt][:],
                        lhsT=wbT_t[:, c * INNER + mt * 128: c * INNER + (mt + 1) * 128],
                        rhs=featT[:, c * 256 + g * 128: c * 256 + (g + 1) * 128],
                        start=(c == HC), stop=False,
                    )

        # --- PE: group-B masked mean (diag stationaries, PSUM-averaged) ---
        psa = pst.tile([128, 512], F32, tag="psa", bufs=1)
        psb = pst.tile([128, 256], F32, tag="psb", bufs=1)
        for j in range(JB):
            if j < J1B:
                src = gB1[:, j * H:(j + 1) * H]
            else:
                src = gB2[:, (j - J1B) * H:(j - J1B + 1) * H]
            dm = dmaskB[:, j * 128:(j + 1) * 128]
            nc.tensor.matmul(psa[:], lhsT=dm, rhs=src[:, 0:512],
                             start=(j == 0), stop=(j == JB - 1))
            nc.tensor.matmul(psb[:], lhsT=dm, rhs=src[:, 512:H],
                             start=(j == 0), stop=(j == JB - 1))

        # --- group-A masked mean: GpSimd tensor_scalar mults (Pool engine
        # is free after descriptor gen) + alternating Vector/GpSimd pairwise
        # add tree.  The V9 strided tensor_reduce took 6.6us - never again. ---
        mgA = pool.tile([128, JA * H], F16)
        for j in range(JA):
            if j < J1A:
                srcj = gA1[:, j * H:(j + 1) * H]
            else:
                srcj = gA2[:, (j - J1A) * H:(j - J1A + 1) * H]
            nc.gpsimd.tensor_scalar(mgA[:, j * H:(j + 1) * H], srcj,
                                    c32_t[:, MOFF + j:MOFF + j + 1], None,
                                    op0=MUL)

        # scalar drains B's PSUM mean (psa); vector drains psb
        ctB = pool.tile([128, H], F16)
        nc.scalar.copy(ctB[:, 0:512], psa[:])
        nc.vector.tensor_copy(ctB[:, 512:H], psb[:])

        # pairwise add tree over mgA rows, alternating engines
        ctA = pool.tile([128, H], F16)
        engs = [nc.vector, nc.gpsimd]
        level = [mgA[:, j * H:(j + 1) * H] for j in range(JA)]
        tmps = 0
        ei = 0
        while len(level) > 1:
            nxt = []
            for k in range(0, len(level) - 1, 2):
                if len(level) == 2:
                    dst = ctA[:]
                else:
                    t = pool.tile([128, H], F16, name=f"tA{tmps}")
                    tmps += 1
                    dst = t[:]
                engs[ei % 2].tensor_tensor(out=dst, in0=level[k],
                                           in1=level[k + 1], op=ADD)
                ei += 1
                nxt.append(dst)
            if len(level) % 2:
                nxt.append(level[-1])
            level = nxt
        if JA == 1:
            nc.vector.tensor_copy(ctA[:], mgA[:, 0:H])

        featT_pairs = featT[:].rearrange("p (c x) -> p c x", x=256)
        hiddenT = pool.tile([128, 2 * 256], F16)
        out3 = pool.tile([128, 2 * NB_LABELS], F32)  # [p, g*3 + n]
        tpc = pst.tile([128, HC * 128], F16, tag="tpc", bufs=1)

        # g=0 is group B (processed first), g=1 group A
        for g, ct in ((0, ctB), (1, ctA)):
            # center transposes (PE transpose mode, identity permutation)
            for cc in range(HC):
                nc.tensor.transpose(tpc[:, cc * 128:(cc + 1) * 128],
                                    ct[:, cc * 128:(cc + 1) * 128], identity)
            tpcv = tpc[:].rearrange("p (c x) -> p c x", c=HC)
            nc.scalar.copy(featT_pairs[:, 0:3, g * 128:(g + 1) * 128],
                           tpcv[:, 0:3, :])
            nc.vector.tensor_copy(featT_pairs[:, 3:HC, g * 128:(g + 1) * 128],
                                  tpcv[:, 3:HC, :])

            # center chunks close the base-linear accumulation; bias+relu
            for c in range(HC):
                for mt in range(2):
                    nc.tensor.matmul(
                        accs[g][mt][:],
                        lhsT=wbT_t[:, c * INNER + mt * 128: c * INNER + (mt + 1) * 128],
                        rhs=featT[:, c * 256 + g * 128: c * 256 + (g + 1) * 128],
                        start=False, stop=(c == HC - 1),
                    )
            for mt in range(2):
                nc.scalar.activation(
                    hiddenT[:, mt * 256 + g * 128: mt * 256 + (g + 1) * 128],
                    accs[g][mt][:],
                    mybir.ActivationFunctionType.Relu,
                    bias=bb_t[:, mt:mt + 1], scale=1.0)

        for g in range(2):
            # expert heads + per-sample selection; ps36 reuses drained psa/psb
            b0 = g * 128
            mask36 = pool.tile([128, NE], F32, tag=f"mask36{g}", bufs=1)
            nc.vector.tensor_scalar(mask36[:], io36f, catf[:, g:g + 1], None,
                                    op0=mybir.AluOpType.is_equal)
            pe = (psa if g == 0 else psb)[:, 0:NE]
            nc.tensor.matmul(pe, lhsT=hiddenT[:, b0:b0 + 128],
                             rhs=wexpA, start=True, stop=False)
            nc.tensor.matmul(pe, lhsT=hiddenT[:, 256 + b0:256 + b0 + 128],
                             rhs=wexpB, start=False, stop=False)
            nc.tensor.matmul(pe, lhsT=ones1[0:1, b0:b0 + 128],
                             rhs=wexpC, start=False, stop=True)

            prod = pool.tile([128, NE], F32, tag=f"prod{g}", bufs=1)
            nc.vector.tensor_tensor(out=prod[:], in0=pe, in1=mask36[:], op=MUL)
            nc.vector.tensor_reduce(
                out=out3[:, g * NB_LABELS:(g + 1) * NB_LABELS],
                in_=prod[:].rearrange("p (e n) -> p n e", n=NB_LABELS),
                axis=mybir.AxisListType.X, op=ADD)

        nc.sync.dma_start(out[:, :], out3[:])

    nc.compile()
    return nc


_NC = {}


def _get_nc(JA, JB):
    key = (JA, JB)
    if key not in _NC:
        _NC[key] = _build(JA, JB)
    return _NC[key]


def _prep_inputs(embeddings, position_indexes, categories, W_base, b_base,
                 W_experts, b_experts):
    emb32 = np.asarray(embeddings)
    emb16 = emb32.astype(np.float16).reshape(NCORES, BC, S, H)

    pos = np.asarray(position_indexes).astype(np.int64).reshape(NCORES, BC, 2)
    cat = np.asarray(categories).astype(np.int64).reshape(NCORES, BC)

    lens_all = pos[:, :, 1] - pos[:, :, 0]                     # [NC, 256]
    perm = np.argsort(lens_all, axis=1, kind="stable")         # [NC, 256]
    # group order: B (longest 128) first = g0, then A (shortest) = g1
    permP = np.concatenate([perm[:, 128:], perm[:, :128]], axis=1)
    lensP = np.take_along_axis(lens_all, permP, 1)
    startsP = np.take_along_axis(pos[:, :, 0], permP, 1)
    catP = np.take_along_axis(cat, permP, 1)

    JB = int(lensP[:, :128].max())
    JA = int(lensP[:, 128:].max())
    assert 1 <= JA <= 8 and 1 <= JB <= 8
    J1A, J1B = (JA + 1) // 2, (JB + 1) // 2

    row = permP * S + startsP                                  # [NC, 256]
    gidx = np.empty((NCORES, 128, 4), dtype=np.int32)
    gidx[:, :, 0] = row[:, :128]
    gidx[:, :, 1] = row[:, :128] + J1B
    gidx[:, :, 2] = row[:, 128:]
    gidx[:, :, 3] = row[:, 128:] + J1A

    # base linear: wbT[p, c*INNER+m] = W_base[m, c*128+p], shipped contiguous
    wb = np.asarray(W_base, dtype=np.float32)  # [INNER, 3H]
    wbT = np.ascontiguousarray(
        wb.T.reshape(KC, 128, INNER).transpose(1, 0, 2).reshape(128, KC * INNER)
    ).astype(np.float16)

    bbias = np.asarray(b_base, dtype=np.float32)

    we = np.asarray(W_experts, dtype=np.float32)  # [12, 3, INNER]
    be = np.asarray(b_experts, dtype=np.float32)  # [12, 3]
    wexp = we.transpose(2, 0, 1).reshape(INNER, NE)
    eye = np.eye(128, dtype=np.float32)
    lensB = lensP[:, :128].astype(np.float32)                  # [NC, 128]
    diagB = (eye[None] * (1.0 / lensB)[:, :, None]).astype(np.float16)
    c16 = np.concatenate(
        [diagB, np.broadcast_to(eye[None].astype(np.float16), (NCORES, 128, 128)),
         np.broadcast_to(wexp[None, 0:128].astype(np.float16), (NCORES, 128, NE)),
         np.broadcast_to(wexp[None, 128:256].astype(np.float16), (NCORES, 128, NE))],
        axis=2)
    c1 = np.concatenate(
        [np.ones((1, 256), dtype=np.float32), be.reshape(1, NE)],
        axis=1).astype(np.float16)

    # static context rows in featT layout, permP order:
    # ctxT[p, (w*HC+cc)*256 + g*128 + sl] = emb[permP[g*128+sl], CTX_IDX[w], cc*128+p]
    blocks = []
    for which in range(NB_CTX):
        blk = emb16[:, :, CTX_IDX[which], :]                   # [NC, 256, 768]
        blkP = np.take_along_axis(blk, permP[:, :, None], 1)
        arr = blkP.reshape(NCORES, 2, 128, HC, 128).transpose(0, 4, 3, 1, 2)
        blocks.append(arr.reshape(NCORES, 128, HC * 256))
    ctxT = np.ascontiguousarray(np.concatenate(blocks, axis=2))

    # io36 + categories-as-float + mA (1/len-scaled) + mB (0/1) + b_base
    MOFF = NE + 2
    cst32 = np.zeros((NCORES, 128, MOFF + JA + JB + 2), dtype=np.float32)
    cst32[:, :, :NE] = np.repeat(np.arange(NB_EXPERTS, dtype=np.float32),
                                 NB_LABELS)[None, None, :]
    cst32[:, :, NE:NE + 2] = catP.reshape(NCORES, 2, 128).transpose(0, 2, 1)
    lensA = lensP[:, 128:].astype(np.float32)
    j = np.arange(JA, dtype=np.float32)
    cst32[:, :, MOFF:MOFF + JA] = (
        (j[None, None, :] < lensA[:, :, None]) / lensA[:, :, None])
    j = np.arange(JB, dtype=np.float32)
    cst32[:, :, MOFF + JA:MOFF + JA + JB] = (
        j[None, None, :] < lensB[:, :, None]).astype(np.float32)
    cst32[:, :, MOFF + JA + JB:] = bbias.reshape(2, 128).T[None]

    in_maps = [
        {"embT": np.ascontiguousarray(emb16[i].reshape(EROWS, H)),
         "gidx": np.ascontiguousarray(gidx[i]),
         "wbT": wbT, "ctxT": ctxT[i],
         "c16": np.ascontiguousarray(c16[i]),
         "c32": np.ascontiguousarray(cst32[i]),
         "c1": np.ascontiguousarray(c1)}
        for i in range(NCORES)
    ]
    return {"in_maps": in_maps, "perm": permP, "key": (JA, JB)}


def _run(prep, **kw):
    nc = _get_nc(*prep["key"])
    return run_bass_kernel_spmd(nc, prep["in_maps"],
                                core_ids=list(range(NCORES)), **kw)


def _postprocess(prep, res):
    perm = prep["perm"]
    full = np.empty((B, NB_LABELS), dtype=np.float32)
    for i, r in enumerate(res.results):
        arr = r["out"].reshape(128, 2, NB_LABELS).transpose(1, 0, 2).reshape(
            BC, NB_LABELS)
        full[i * BC + perm[i]] = arr
    return full


def kernel(embeddings, position_indexes, categories, W_base, b_base, W_experts,
           b_experts):
    prep = _prep_inputs(embeddings, position_indexes, categories, W_base,
                        b_base, W_experts, b_experts)
    res = _run(prep)
    return _postprocess(prep, res)


# revision 8
# speedup vs baseline: 2.3535x; 2.3535x over previous
"""Trainium2 Bass kernel for nn_BertClassifier_77309411685 (V9).

Data-parallel over 8 NeuronCores: each core handles 256 samples; the small
base linear and 12 expert heads are replicated.

V10 strategy (V8/V9 trace forensics: DVE scalar_tensor_tensor runs at ~1us per
768-elem op - 4x slower than tensor_scalar - so the all-Vector masked-mean
chain serialized 14us; DMA engines serve queues roughly FIFO at ~400GB/s
aggregate; indirect-DMA descriptor gen costs ~1.1us per op on GpSimd):
  * samples per core are permuted by span length (host-side; un-permuted on
    host): group B (g=0) = 128 longest spans (JB rows), group A (g=1) =
    128 shortest (JA rows).  Cuts gather bytes ~20% and PE mean work.
  * group B mean on the PE (V7 diag-stationary scheme, 0.84ns/col - the
    fastest engine per byte); group A mean on Vector (tensor_scalar mults
    + one strided tensor_reduce over j).  This splits the ~8.4us of mean
    work across engines so the PE (the bottleneck at ~16us) stays fed.
  * wire order: ctx weights first (feeds ctx matmuls), B gather (feeds the
    long PE mean chain), A gather, wbT-center LAST (its consumers - the
    close matmuls - run last anyway; a dummy sync DMA reading gB2 delays
    its issue so it cannot steal wire from B/A).
  * wbT host-prearranged [128, KC*INNER] so weight DMAs are contiguous
    multi-KB descriptors; gathers use 2 pieces per group (6KB descriptors).
  * PE warm-up from a memset tile during the DMA ramp (HAM clock gate).
  * expert heads: all 12 experts at once, bias via ones row; per-sample
    is_equal select + strided reduce; one packed [128, 6] output DMA.
"""

import numpy as np
from contextlib import ExitStack

import concourse.bass as bass
import concourse.tile as tile
from concourse import bacc, mybir
from concourse.bass import IndirectOffsetOnAxis
from concourse.bass_utils import run_bass_kernel_spmd

F32 = mybir.dt.float32
F16 = mybir.dt.float16
I32 = mybir.dt.int32

B, S, H = 2048, 256, 768
INNER, NB_CTX, NB_EXPERTS, NB_LABELS = 256, 2, 12, 3
NCORES = 8
BC = B // NCORES             # 256 samples per core
F3H = (NB_CTX + 1) * H       # 2304
KC = F3H // 128              # 18 contraction chunks
HC = H // 128                # 6 chunks per feature block
NE = NB_EXPERTS * NB_LABELS  # 36
EROWS = BC * S               # rows in the per-core embedding tensor

# The reference picks 2 static context positions host-side with this exact rng.
CTX_IDX = [int(v) for v in np.random.default_rng(seed=0).choice(np.arange(S), size=NB_CTX)]

MUL = mybir.AluOpType.mult
ADD = mybir.AluOpType.add


def _build(JA, JB):
    """Build the per-core program for group row counts (JA, JB)."""
    J1A, J1B = (JA + 1) // 2, (JB + 1) // 2
    J2A, J2B = JA - J1A, JB - J1B
    MOFF = NE + 2                                # mask cols offset in c32

    nc = bacc.Bacc(
        "TRN2",
        target_bir_lowering=False,
        debug=False,
        enable_asserts=False,
        num_devices=NCORES,
    )
    embT = nc.dram_tensor("embT", [EROWS, H], F16, kind="ExternalInput").ap()
    # gidx cols: 0=B rows [0,J1B), 1=B rows [J1B,JB), 2=A rows [0,J1A),
    #            3=A rows [J1A,JA)
    gidx = nc.dram_tensor("gidx", [128, 4], I32, kind="ExternalInput").ap()
    # wbT pre-arranged: wbT[p, c*INNER + m] = W_base[m, c*128 + p]
    wbT = nc.dram_tensor("wbT", [128, KC * INNER], F16, kind="ExternalInput").ap()
    ctxT = nc.dram_tensor("ctxT", [128, NB_CTX * HC * 256], F16, kind="ExternalInput").ap()
    # c16: diag(1/len_B) [0:128) + identity [128:256) + wexpA + wexpB
    c16 = nc.dram_tensor("c16", [128, 256 + 2 * NE], F16, kind="ExternalInput").ap()
    # c32: io36 + categories-as-float (2) + mA*1/len (JA) + mB 0/1 (JB)
    #      + b_base (t p) layout (2)
    c32 = nc.dram_tensor("c32", [128, MOFF + JA + JB + 2], F32, kind="ExternalInput").ap()
    # c1: ones row [0:256) + expert bias row [256:292)
    c1 = nc.dram_tensor("c1", [1, 256 + NE], F16, kind="ExternalInput").ap()
    out = nc.dram_tensor("out", [128, 2 * NB_LABELS], F32, kind="ExternalOutput").ap()

    with tile.TileContext(nc) as tc, ExitStack() as ctx:
        pool = ctx.enter_context(tc.tile_pool(name="main", bufs=1))
        pst = ctx.enter_context(tc.tile_pool(name="pst", bufs=1, space="PSUM"))

        # --- tiny front-of-queue loads ---
        # gidx rides the Scalar queue FIRST: the gpsimd queue's tiny DMA
        # took ~4us to semaphore behind the jammed weight stream.
        gidx_t = pool.tile([128, 4], I32)
        nc.scalar.dma_start(gidx_t[:], gidx[:, :])

        c16_t = pool.tile([128, 256 + 2 * NE], F16)
        nc.scalar.dma_start(c16_t[:], c16[:, :])
        diagB = c16_t[:, 0:128]
        identity = c16_t[:, 128:256]
        wexpA = c16_t[:, 256:256 + NE]
        wexpB = c16_t[:, 256 + NE:256 + 2 * NE]
        c32_t = pool.tile([128, MOFF + JA + JB + 2], F32)
        nc.scalar.dma_start(c32_t[:], c32[:, :])
        io36f = c32_t[:, 0:NE]
        catf = c32_t[:, NE:NE + 2]
        bb_t = c32_t[:, MOFF + JA + JB:MOFF + JA + JB + 2]
        c1_t = pool.tile([1, 256 + NE], F16)
        nc.scalar.dma_start(c1_t[:], c1[:, :])
        ones1 = c1_t[:, 0:256]
        wexpC = c1_t[:, 256:256 + NE]

        # PE warm-up from a memset tile (no DMA dependency)
        warm_src = pool.tile([128, 512], F16)
        nc.vector.memset(warm_src[:], 0.0)
        warm = pst.tile([128, 512], F32, tag="warm", bufs=1)
        for w in range(3):
            nc.tensor.matmul(warm[:], lhsT=warm_src[:, 0:128], rhs=warm_src[:],
                             start=(w == 0), stop=(w == 2))

        # --- span gathers on the gpsimd SWDGE queue: B first (feeds the
        # long PE chain), A after ---
        def gather(col, rows, tag):
            gt = pool.tile([128, rows * H], F16, tag=tag, bufs=1)
            nc.gpsimd.indirect_dma_start(
                out=gt[:], out_offset=None, in_=embT,
                in_offset=IndirectOffsetOnAxis(ap=gidx_t[:, col:col + 1], axis=0),
            )
            return gt

        gB1 = gather(0, J1B, "gB1")
        gB2 = gather(1, J2B, "gB2") if J2B else None
        gA1 = gather(2, J1A, "gA1")
        gA2 = gather(3, J2A, "gA2") if J2A else None

        # --- weight streams on the Sync queue: ctx parts first; wbT-center
        # delayed behind a dummy DMA that reads gB2 so it issues only after
        # the B gather has drained the wire ---
        featT = pool.tile([128, KC * 256], F16)
        wbT_t = pool.tile([128, KC * INNER], F16)
        nc.sync.dma_start(wbT_t[:, HC * INNER:], wbT[:, HC * INNER:])
        nc.sync.dma_start(featT[:, HC * 256:], ctxT[:, :])
        nc.sync.dma_start(wbT_t[:, :HC * INNER], wbT[:, :HC * INNER])

        # --- Vector: build diag(mB_j/len) stationaries for the PE mean ---
        dmaskB = pool.tile([128, JB * 128], F16)
        for j in range(JB):
            nc.vector.tensor_scalar(
                dmaskB[:, j * 128:(j + 1) * 128], diagB,
                c32_t[:, MOFF + JA + j:MOFF + JA + j + 1], None, op0=MUL)

        # --- PE: ctx chunks (paced by the weight stream) ---
        accs = [[pst.tile([128, 128], F32, tag=f"acc{g}{mt}", bufs=1,
                          name=f"acc{g}{mt}") for mt in range(2)]
                for g in range(2)]
        for c in range(HC, KC):
            for g in range(2):
                for mt in range(2):
                    nc.tensor.matmul(
                        accs[g][mt][:],
                        lhsT=wbT_t[:, c * INNER + mt * 128: c * INNER + (mt + 1) * 128],
                        rhs=featT[:, c * 256 + g * 128: c * 256 + (g + 1) * 128],
                        start=(c == HC), stop=False,
                    )

        # --- PE: group-B masked mean (diag stationaries, PSUM-averaged) ---
        psa = pst.tile([128, 512], F32, tag="psa", bufs=1)
        psb = pst.tile([128, 256], F32, tag="psb", bufs=1)
        for j in range(JB):
            if j < J1B:
                src = gB1[:, j * H:(j + 1) * H]
            else:
                src = gB2[:, (j - J1B) * H:(j - J1B + 1) * H]
            dm = dmaskB[:, j * 128:(j + 1) * 128]
            nc.tensor.matmul(psa[:], lhsT=dm, rhs=src[:, 0:512],
                             start=(j == 0), stop=(j == JB - 1))
            nc.tensor.matmul(psb[:], lhsT=dm, rhs=src[:, 512:H],
                             start=(j == 0), stop=(j == JB - 1))

        # --- group-A masked mean: GpSimd tensor_scalar mults (Pool engine
        # is free after descriptor gen) + alternating Vector/GpSimd pairwise
        # add tree.  The V9 strided tensor_reduce took 6.6us - never again. ---
        mgA = pool.tile([128, JA * H], F16)
        for j in range(JA):
            if j < J1A:
                srcj = gA1[:, j * H:(j + 1) * H]
            else:
                srcj = gA2[:, (j - J1A) * H:(j - J1A + 1) * H]
            nc.vector.tensor_scalar(mgA[:, j * H:(j + 1) * H], srcj,
                                    c32_t[:, MOFF + j:MOFF + j + 1], None,
                                    op0=MUL)

        # scalar drains B's PSUM mean (psa); vector drains psb
        ctB = pool.tile([128, H], F16)
        nc.scalar.copy(ctB[:, 0:512], psa[:])
        nc.vector.tensor_copy(ctB[:, 512:H], psb[:])

        # pairwise add tree over mgA rows: Vector TT is 0.56us; Pool TT
        # (1.6us) takes alternating level-1 branches only (Pool TS is 11us -
        # never use it)
        ctA = pool.tile([128, H], F16)
        level = [mgA[:, j * H:(j + 1) * H] for j in range(JA)]
        tmps = 0
        first_level = True
        while len(level) > 1:
            nxt = []
            for k in range(0, len(level) - 1, 2):
                if len(level) == 2:
                    dst = ctA[:]
                else:
                    t = pool.tile([128, H], F16, name=f"tA{tmps}")
                    tmps += 1
                    dst = t[:]
                eng = nc.gpsimd if (first_level and k == 2) else nc.vector
                eng.tensor_tensor(out=dst, in0=level[k], in1=level[k + 1],
                                  op=ADD)
                nxt.append(dst)
            if len(level) % 2:
                nxt.append(level[-1])
            level = nxt
            first_level = False
        if JA == 1:
            nc.vector.tensor_copy(ctA[:], mgA[:, 0:H])

        featT_pairs = featT[:].rearrange("p (c x) -> p c x", x=256)
        hiddenT = pool.tile([128, 2 * 256], F16)
        out3 = pool.tile([128, 2 * NB_LABELS], F32)  # [p, g*3 + n]
        tpc = pst.tile([128, HC * 128], F16, tag="tpc", bufs=1)

        # g=0 is group B (processed first), g=1 group A
        for g, ct in ((0, ctB), (1, ctA)):
            # center transposes (PE transpose mode, identity permutation)
            for cc in range(HC):
                nc.tensor.transpose(tpc[:, cc * 128:(cc + 1) * 128],
                                    ct[:, cc * 128:(cc + 1) * 128], identity)
            tpcv = tpc[:].rearrange("p (c x) -> p c x", c=HC)
            nc.scalar.copy(featT_pairs[:, 0:3, g * 128:(g + 1) * 128],
                           tpcv[:, 0:3, :])
            nc.vector.tensor_copy(featT_pairs[:, 3:HC, g * 128:(g + 1) * 128],
                                  tpcv[:, 3:HC, :])

            # center chunks close the base-linear accumulation; bias+relu
            for c in range(HC):
                for mt in range(2):
                    nc.tensor.matmul(
                        accs[g][mt][:],
                        lhsT=wbT_t[:, c * INNER + mt * 128: c * INNER + (mt + 1) * 128],
                        rhs=featT[:, c * 256 + g * 128: c * 256 + (g + 1) * 128],
                        start=False, stop=(c == HC - 1),
                    )
            for mt in range(2):
                nc.scalar.activation(
                    hiddenT[:, mt * 256 + g * 128: mt * 256 + (g + 1) * 128],
                    accs[g][mt][:],
                    mybir.ActivationFunctionType.Relu,
                    bias=bb_t[:, mt:mt + 1], scale=1.0)

        for g in range(2):
            # expert heads + per-sample selection; ps36 reuses drained psa/psb
            b0 = g * 128
            mask36 = pool.tile([128, NE], F32, tag=f"mask36{g}", bufs=1)
            nc.vector.tensor_scalar(mask36[:], io36f, catf[:, g:g + 1], None,
                                    op0=mybir.AluOpType.is_equal)
            pe = (psa if g == 0 else psb)[:, 0:NE]
            nc.tensor.matmul(pe, lhsT=hiddenT[:, b0:b0 + 128],
                             rhs=wexpA, start=True, stop=False)
            nc.tensor.matmul(pe, lhsT=hiddenT[:, 256 + b0:256 + b0 + 128],
                             rhs=wexpB, start=False, stop=False)
            nc.tensor.matmul(pe, lhsT=ones1[0:1, b0:b0 + 128],
                             rhs=wexpC, start=False, stop=True)

            prod = pool.tile([128, NE], F32, tag=f"prod{g}", bufs=1)
            nc.vector.tensor_tensor(out=prod[:], in0=pe, in1=mask36[:], op=MUL)
            nc.vector.tensor_reduce(
                out=out3[:, g * NB_LABELS:(g + 1) * NB_LABELS],
                in_=prod[:].rearrange("p (e n) -> p n e", n=NB_LABELS),
                axis=mybir.AxisListType.X, op=ADD)

        nc.sync.dma_start(out[:, :], out3[:])

    nc.compile()
    return nc


_NC = {}


def _get_nc(JA, JB):
    key = (JA, JB)
    if key not in _NC:
        _NC[key] = _build(JA, JB)
    return _NC[key]


def _prep_inputs(embeddings, position_indexes, categories, W_base, b_base,
                 W_experts, b_experts):
    emb32 = np.asarray(embeddings)
    emb16 = emb32.astype(np.float16).reshape(NCORES, BC, S, H)

    pos = np.asarray(position_indexes).astype(np.int64).reshape(NCORES, BC, 2)
    cat = np.asarray(categories).astype(np.int64).reshape(NCORES, BC)

    lens_all = pos[:, :, 1] - pos[:, :, 0]                     # [NC, 256]
    perm = np.argsort(lens_all, axis=1, kind="stable")         # [NC, 256]
    # group order: B (longest 128) first = g0, then A (shortest) = g1
    permP = np.concatenate([perm[:, 128:], perm[:, :128]], axis=1)
    lensP = np.take_along_axis(lens_all, permP, 1)
    startsP = np.take_along_axis(pos[:, :, 0], permP, 1)
    catP = np.take_along_axis(cat, permP, 1)

    JB = int(lensP[:, :128].max())
    JA = int(lensP[:, 128:].max())
    assert 1 <= JA <= 8 and 1 <= JB <= 8
    J1A, J1B = (JA + 1) // 2, (JB + 1) // 2

    row = permP * S + startsP                                  # [NC, 256]
    gidx = np.empty((NCORES, 128, 4), dtype=np.int32)
    gidx[:, :, 0] = row[:, :128]
    gidx[:, :, 1] = row[:, :128] + J1B
    gidx[:, :, 2] = row[:, 128:]
    gidx[:, :, 3] = row[:, 128:] + J1A

    # base linear: wbT[p, c*INNER+m] = W_base[m, c*128+p], shipped contiguous
    wb = np.asarray(W_base, dtype=np.float32)  # [INNER, 3H]
    wbT = np.ascontiguousarray(
        wb.T.reshape(KC, 128, INNER).transpose(1, 0, 2).reshape(128, KC * INNER)
    ).astype(np.float16)

    bbias = np.asarray(b_base, dtype=np.float32)

    we = np.asarray(W_experts, dtype=np.float32)  # [12, 3, INNER]
    be = np.asarray(b_experts, dtype=np.float32)  # [12, 3]
    wexp = we.transpose(2, 0, 1).reshape(INNER, NE)
    eye = np.eye(128, dtype=np.float32)
    lensB = lensP[:, :128].astype(np.float32)                  # [NC, 128]
    diagB = (eye[None] * (1.0 / lensB)[:, :, None]).astype(np.float16)
    c16 = np.concatenate(
        [diagB, np.broadcast_to(eye[None].astype(np.float16), (NCORES, 128, 128)),
         np.broadcast_to(wexp[None, 0:128].astype(np.float16), (NCORES, 128, NE)),
         np.broadcast_to(wexp[None, 128:256].astype(np.float16), (NCORES, 128, NE))],
        axis=2)
    c1 = np.concatenate(
        [np.ones((1, 256), dtype=np.float32), be.reshape(1, NE)],
        axis=1).astype(np.float16)

    # static context rows in featT layout, permP order:
    # ctxT[p, (w*HC+cc)*256 + g*128 + sl] = emb[permP[g*128+sl], CTX_IDX[w], cc*128+p]
    blocks = []
    for which in range(NB_CTX):
        blk = emb16[:, :, CTX_IDX[which], :]                   # [NC, 256, 768]
        blkP = np.take_along_axis(blk, permP[:, :, None], 1)
        arr = blkP.reshape(NCORES, 2, 128, HC, 128).transpose(0, 4, 3, 1, 2)
        blocks.append(arr.reshape(NCORES, 128, HC * 256))
    ctxT = np.ascontiguousarray(np.concatenate(blocks, axis=2))

    # io36 + categories-as-float + mA (1/len-scaled) + mB (0/1) + b_base
    MOFF = NE + 2
    cst32 = np.zeros((NCORES, 128, MOFF + JA + JB + 2), dtype=np.float32)
    cst32[:, :, :NE] = np.repeat(np.arange(NB_EXPERTS, dtype=np.float32),
                                 NB_LABELS)[None, None, :]
    cst32[:, :, NE:NE + 2] = catP.reshape(NCORES, 2, 128).transpose(0, 2, 1)
    lensA = lensP[:, 128:].astype(np.float32)
    j = np.arange(JA, dtype=np.float32)
    cst32[:, :, MOFF:MOFF + JA] = (
        (j[None, None, :] < lensA[:, :, None]) / lensA[:, :, None])
    j = np.arange(JB, dtype=np.float32)
    cst32[:, :, MOFF + JA:MOFF + JA + JB] = (
        j[None, None, :] < lensB[:, :, None]).astype(np.float32)
    cst32[:, :, MOFF + JA + JB:] = bbias.reshape(2, 128).T[None]

    in_maps = [
        {"embT": np.ascontiguousarray(emb16[i].reshape(EROWS, H)),
         "gidx": np.ascontiguousarray(gidx[i]),
         "wbT": wbT, "ctxT": ctxT[i],
         "c16": np.ascontiguousarray(c16[i]),
         "c32": np.ascontiguousarray(cst32[i]),
         "c1": np.ascontiguousarray(c1)}
        for i in range(NCORES)
    ]
    return {"in_maps": in_maps, "perm": permP, "key": (JA, JB)}


def _run(prep, **kw):
    nc = _get_nc(*prep["key"])
    return run_bass_kernel_spmd(nc, prep["in_maps"],
                                core_ids=list(range(NCORES)), **kw)


def _postprocess(prep, res):
    perm = prep["perm"]
    full = np.empty((B, NB_LABELS), dtype=np.float32)
    for i, r in enumerate(res.results):
        arr = r["out"].reshape(128, 2, NB_LABELS).transpose(1, 0, 2).reshape(
            BC, NB_LABELS)
        full[i * BC + perm[i]] = arr
    return full


def kernel(embeddings, position_indexes, categories, W_base, b_base, W_experts,
           b_experts):
    prep = _prep_inputs(embeddings, position_indexes, categories, W_base,
                        b_base, W_experts, b_experts)
    res = _run(prep)
    return _postprocess(prep, res)
